# revision 34
# baseline (speedup 1.0000x reference)
"""Baichuan attention on 8 Trainium2 NeuronCores — tensor-parallel over heads.

Sharding: core c computes heads [4c, 4c+4): its slice of the fused QKV
projection, attention for those heads, then 1/8 of o_proj's output columns
after an AllGather of the per-core context slices (moves 4MB/rank instead of
a 32MB AllReduce of partial sums; mathematically identical to the module's
world_size logic).

Schedule (v2): attention blocks are interleaved into the QKV phase so the
collectives and o_proj always have PE work to hide behind:

    half0-QKV | att(1) | half1-QKV | att(2) op(1) att(3) op(2)
                                     att(0) op(3) op(0)

att(1) runs between the QKV halves (hiding the half1 x-prefetch), att(0) is
deferred to the end so the final o_proj's AllGather completes while two other
o_proj blocks stream — the serial AllGather+o_proj tail of v1 disappears.

Causal structure: diagonal score tiles are narrowed to their live columns
(moving width 512-128j) and masked with a single resident 128x128 triangular
constant instead of a 4MB mask DMA. Score tiles are computed in pairs into
2-bank PSUM tiles so one exp activation covers both (halving ACT dispatch
overhead); av/rowsum matmuls lag one slot behind scores, which hides the
scores->exp->av chain latency at a steady state of zero PE bubbles.

Matmul operands are fp16 (1 cyc/row on the PE) with fp32 PSUM accumulation.
"""

import numpy as np

import concourse.bacc as bacc
import concourse.bass_isa as bass_isa
import concourse.mybir as mybir
import concourse.tile as tile
from concourse.bass_utils import run_bass_kernel_spmd

F32 = mybir.dt.float32

N_CORES = 8
NUM_HEADS = 32
HEAD_DIM = 128
P = 128          # SBUF partitions / PE contraction tile
SQ = 512         # s_q block width (PSUM bank = 512 fp32)
MM_MODE = "f16"  # 'f16' | 'f32' (operand dtype for matmuls)

_CACHE: dict = {}


def _mm_dtype(mode):
    return {"f16": mybir.dt.float16, "f32": F32}[mode]


def build(S, H, mode=MM_MODE):
    MD = _mm_dtype(mode)
    hpc = NUM_HEADS // N_CORES          # heads per core
    dpc = hpc * HEAD_DIM                # per-core slice of the hidden dim
    n_ht = H // P                       # contraction tiles for QKV/o_proj
    n_qk = 2 * dpc // P                 # q+k output tiles
    n_sq = S // SQ                      # s_q blocks
    n_st = S // P                       # s_k tiles
    scale = 1.0 / np.sqrt(np.float32(HEAD_DIM))
    s_half = S // 2
    sb_per_half = s_half // SQ
    diag_per_b = SQ // P                # diagonal k-tiles per s_q block

    nc = bacc.Bacc("TRN2", target_bir_lowering=False, debug=False,
                   num_devices=N_CORES)

    xT = nc.dram_tensor("xT", [H, S], MD, kind="ExternalInput")
    wqkT = nc.dram_tensor("wqkT", [H, 2 * dpc], MD, kind="ExternalInput")
    wvT = nc.dram_tensor("wvT", [H, dpc], MD, kind="ExternalInput")
    tri = nc.dram_tensor("tri", [P, P], F32, kind="ExternalInput")
    woT = nc.dram_tensor("woT", [H, dpc], MD, kind="ExternalInput")
    out_cols = nc.dram_tensor("out_cols", [S, dpc], F32, kind="ExternalOutput")

    # AllGather in head-pair chunks: gat[b][pp] holds local heads
    # {2pp, 2pp+1} for s_q block b; ct[b][pp] gathers those pairs from all
    # ranks. o_proj consumes them against host-permuted w_o rows.
    gat_b = [[nc.dram_tensor(f"gat_{b}_{pp}", [dpc // 2, SQ], MD)
              for pp in range(2)] for b in range(n_sq)]
    ct_b = [[nc.dram_tensor(f"ct_{b}_{pp}", [H // 2, SQ], MD,
                            addr_space="Shared") for pp in range(2)]
            for b in range(n_sq)]

    xT_t = xT.ap().rearrange("(t p) s -> p t s", p=P)
    wqkT_t = wqkT.ap().rearrange("(t p) o -> p t o", p=P)
    wvT_t = wvT.ap().rearrange("(t p) o -> p t o", p=P)
    woT_t = woT.ap().rearrange("(t p) j -> p t j", p=P)

    with tile.TileContext(nc) as tc:
        with (
            tc.tile_pool(name="consts", bufs=1) as cpool,
            tc.tile_pool(name="span", bufs=1) as span,
            tc.tile_pool(name="qkv_wv", bufs=1) as wvpool,
            tc.tile_pool(name="at_exp", bufs=3) as epool,
            tc.tile_pool(name="at_out", bufs=2) as opool,
            tc.tile_pool(name="at_r", bufs=2) as rpool,
        ):
            ones_s = cpool.tile([P, P], F32, tag="ones_s")
            nc.gpsimd.memset(ones_s[:], 1.0)
            ones_f = cpool.tile([P, P], mybir.dt.float32r, tag="ones_f")
            nc.vector.tensor_copy(ones_f[:], ones_s[:])
            tri_t = cpool.tile([P, P], F32, tag="tri")
            nc.sync.dma_start(tri_t[:], tri.ap()[:, :])

            # v ([s_k, d] natural, all heads) and q/k (transposed, all heads)
            # live in SBUF for the whole kernel; QKV evictions write them
            # directly (no DRAM bounce)
            v_sb = span.tile([P, n_st, dpc], MD, tag="v")
            qk_all = span.tile([P, n_qk, S], MD, tag="qk")
            wv_sb = wvpool.tile([P, n_ht, dpc], MD, tag="wv")

            # =============== QKV projection (one half of S) ===============
            # q/k transposed into resident qk_all; v natural into v_sb.
            def qkv_half(half, xpool, wpool, pspool):
                if half == 0:
                    # wv upfront in 8-t chunks (per-descriptor DMA bandwidth
                    # is limited, so chunking keeps it ahead of the first
                    # v chains)
                    for c in range(n_ht // 8):
                        nc.sync.dma_start(wv_sb[:, c * 8:(c + 1) * 8, :],
                                          wvT_t[:, c * 8:(c + 1) * 8, :])
                # x arrives in 8-t-tile chunks: batching descriptors keeps
                # the sync queue's ~0.7us/descriptor issue rate off the
                # critical path (64 fine-grained descriptors took ~45us to
                # issue, starving the first ot pass)
                XCH = 8
                xq = []
                for sb in range(sb_per_half):
                    x_tile = xpool.tile([P, n_ht, SQ], MD, tag="x",
                                        bufs=2, name="x_tile")
                    lo = half * s_half + sb * SQ
                    for c in range(n_ht // XCH):
                        nc.sync.dma_start(
                            x_tile[:, c * XCH:(c + 1) * XCH, :],
                            xT_t[:, c * XCH:(c + 1) * XCH, lo:lo + SQ])
                    xq.append(x_tile)
                # v first: its matmuls consume x as the STATIONARY operand
                # (32KB per MM via the LDW port, ~150GB/s of HBM demand) so
                # the x stream stays ahead; q/k then stream fully-resident
                # x as the moving operand (which would need ~500GB/s)
                for sti in range(s_half // P):
                    st_g = half * (s_half // P) + sti
                    sb, off = (sti * P) // SQ, (sti * P) % SQ
                    ps_v = pspool.tile([P, dpc], F32, tag="qkv",
                                       name="ps_v")
                    for t in range(n_ht):
                        nc.tensor.matmul(
                            ps_v[:],
                            xq[sb][:, t, off:off + P],
                            wv_sb[:, t, :],
                            start=(t == 0), stop=(t == n_ht - 1))
                    nc.vector.tensor_copy(v_sb[:, st_g, :], ps_v[:])
                w_tiles = {}
                for ot in range(n_qk):
                    if ot not in w_tiles:
                        w_tiles[ot] = wpool.tile([P, n_ht, P], MD,
                                                 tag="w", name="w_tile")
                        nc.sync.dma_start(
                            w_tiles[ot][:],
                            wqkT_t[:, :, ot * P:(ot + 1) * P])
                    w_tile = w_tiles[ot]
                    if half == 0:
                        # t-outer with both quarters' psums open: the first
                        # ot pass consumes x chunks in DMA-arrival order
                        # instead of draining quarter 0 first, which halves
                        # the startup underrun while x is still streaming
                        # in. Needs 4 psum bufs, so half1 (2 bufs, no
                        # startup problem) keeps the sb-inner form.
                        ps_q = [pspool.tile([P, SQ], F32, tag="qkv",
                                            name=f"ps_q{sb}")
                                for sb in range(sb_per_half)]
                        for t in range(n_ht):
                            for sb in range(sb_per_half):
                                nc.tensor.matmul(
                                    ps_q[sb][:],
                                    w_tile[:, t, :],
                                    xq[sb][:, t, :],
                                    start=(t == 0), stop=(t == n_ht - 1))
                        for sb in range(sb_per_half):
                            # fold the softmax scale into q at eviction
                            mul = scale if ot < dpc // P else 1.0
                            lo = half * s_half + sb * SQ
                            nc.scalar.mul(qk_all[:, ot, lo:lo + SQ],
                                          ps_q[sb][:], mul)
                    else:
                        for sb in range(sb_per_half):
                            ps = pspool.tile([P, SQ], F32, tag="qkv")
                            for t in range(n_ht):
                                nc.tensor.matmul(
                                    ps[:],
                                    w_tile[:, t, :],
                                    xq[sb][:, t, :],
                                    start=(t == 0), stop=(t == n_ht - 1))
                            mul = scale if ot < dpc // P else 1.0
                            lo = half * s_half + sb * SQ
                            nc.scalar.mul(qk_all[:, ot, lo:lo + SQ],
                                          ps[:], mul)

            with (
                tc.tile_pool(name="qkv_x0", bufs=1) as xpool0,
                tc.tile_pool(name="qkv_w0", bufs=3) as wpool0,
                tc.tile_pool(name="qkv_ps0", bufs=4, space="PSUM") as psp0,
            ):
                qkv_half(0, xpool0, wpool0, psp0)

            # attention PSUM pools: pss 2x2 banks + out 1 + row 1 = 6 banks,
            # leaving 2 for the half1 QKV pool / later the o_proj pool
            with tc.tile_pool(name="at_ps", bufs=1, space="PSUM") as aps:

                # ======== attention for s_q block b (4 local heads) ========
                # scores are computed transposed (scoresT[k, q]) so the PE
                # contraction dim sits on partitions for every matmul.
                # Diagonal tiles are narrowed to columns [128j, 512) and get
                # the triangular mask strip added in place. Tiles are
                # processed in slots of two; exp of slot k runs while the PE
                # streams slot k+1's scores, and av/rowsum of slot k follow —
                # steady state has no PE bubble.
                def att_block(b):
                    q_lo = b * SQ
                    n_full = b * diag_per_b
                    slots = []
                    for g in range(n_full // 2):
                        slots.append([(2 * g, 0), (2 * g + 1, 0)])
                    for g in range(diag_per_b // 2):
                        slots.append([(n_full + 2 * g, 2 * g),
                                      (n_full + 2 * g + 1, 2 * g + 1)])
                    n_mm = n_full + diag_per_b
                    for h in range(hpc):
                        ps_o = aps.tile([P, SQ], F32, tag="out",
                                        name="ps_o")
                        # softmax denominator: accumulate exp tiles on the
                        # vector engine (fp16 adds into an SBUF fp32
                        # accumulator) and cross-partition-sum on the idle
                        # gpsimd — frees ~36us of PE streaming vs the
                        # all-ones rowsum matmul, and frees a PSUM bank
                        acc = rpool.tile([P, SQ], mybir.dt.float32r,
                                         tag="acc", name="acc")
                        mi = 0

                        def emit_av(pend):
                            nonlocal mi
                            slot, ex2 = pend
                            for ii, (t, j) in enumerate(slot):
                                lo = P * j
                                first, last = mi == 0, mi == n_mm - 1
                                nc.tensor.matmul(
                                    ps_o[:, lo:SQ],
                                    v_sb[:, t, h * P:(h + 1) * P],
                                    ex2[:, ii, lo:SQ],
                                    start=first, stop=last,
                                    skip_group_check=True)
                                if first:
                                    # first tile is always full-width
                                    nc.vector.tensor_copy(
                                        acc[:], ex2[:, ii, :])
                                else:
                                    nc.vector.tensor_add(
                                        acc[:, lo:SQ], acc[:, lo:SQ],
                                        ex2[:, ii, lo:SQ])
                                mi += 1

                        # av/rowsum lag scores by TWO slots: ps2 is freed by
                        # the exp read (not by av), so depth-2 costs no extra
                        # PSUM and the scores->exp->av chain latency is fully
                        # hidden even at block starts — the PE stream has no
                        # bubble for the scheduler to (mis)fill with o_proj
                        # work whose ct tiles aren't on-chip yet.
                        pend = []
                        for slot in slots:
                            ps2 = aps.tile([P, 2, SQ], F32, tag="pss",
                                           bufs=2, name="ps2")
                            ex2 = epool.tile([P, 2, SQ], MD, tag="exp",
                                             name="ex2")
                            is_diag = slot[0][0] >= n_full
                            for ii, (t, j) in enumerate(slot):
                                lo = P * j
                                nc.tensor.matmul(
                                    ps2[:, ii, lo:SQ],
                                    qk_all[:, hpc + h, t * P:(t + 1) * P],
                                    qk_all[:, h, q_lo + lo:q_lo + SQ],
                                    start=True, stop=True)
                                if is_diag:
                                    nc.vector.tensor_add(
                                        ps2[:, ii, lo:lo + P],
                                        ps2[:, ii, lo:lo + P], tri_t[:])
                            if is_diag:
                                for ii, (t, j) in enumerate(slot):
                                    lo = P * j
                                    nc.scalar.activation(
                                        ex2[:, ii, lo:SQ], ps2[:, ii, lo:SQ],
                                        mybir.ActivationFunctionType.Exp)
                            else:
                                nc.scalar.activation(
                                    ex2[:, :, :], ps2[:, :, :],
                                    mybir.ActivationFunctionType.Exp)
                            pend.append((slot, ex2))
                            if len(pend) > 2:
                                emit_av(pend.pop(0))
                        for p in pend:
                            emit_av(p)

                        # cross-partition sum of the exp accumulator in ONE
                        # 512-wide f32r ones-matmul (0.26us) — vs per-tile
                        # rowsum matmuls this cuts the PE's denominator cost
                        # ~8x; f32r truncation (FP22) adds only ~6e-5 rel
                        # error to the denominator
                        ps_row = aps.tile([P, SQ], F32, tag="row",
                                          name="ps_row")
                        nc.tensor.matmul(
                            ps_row[:], ones_f[:], acc[:],
                            start=True, stop=True)
                        recip = rpool.tile([P, SQ], F32, tag="recip",
                                           name="recip")
                        nc.vector.reciprocal_approx_fast(recip[:], ps_row[:])
                        ob = opool.tile([P, SQ], MD, tag="ob", name="ob")
                        nc.vector.tensor_mul(ob[:], ps_o[:], recip[:])
                        nc.sync.dma_start(
                            gat_b[b][h // 2].ap()[(h % 2) * P:
                                                  (h % 2 + 1) * P, :], ob[:])
                        if h % 2 == 1:
                            nc.gpsimd.collective_compute(
                                "AllGather", mybir.AluOpType.bypass,
                                replica_groups=[list(range(N_CORES))],
                                ins=[gat_b[b][h // 2].ap().opt()],
                                outs=[ct_b[b][h // 2].ap().opt()])

                # att(1) fits between the QKV halves: it needs only the
                # first-half k/v/q, and its PE work hides the half1 x DMA
                with (
                    tc.tile_pool(name="qkv_x1", bufs=1) as xpool1,
                    tc.tile_pool(name="qkv_w1", bufs=3) as wpool1,
                    tc.tile_pool(name="qkv_ps1", bufs=2,
                                 space="PSUM") as psp1,
                ):
                    att_block(1)
                    qkv_half(1, xpool1, wpool1, psp1)

                with (
                    tc.tile_pool(name="op_w", bufs=1) as owpool,
                    tc.tile_pool(name="op_ct", bufs=34) as ctpool,
                    tc.tile_pool(name="op_stage", bufs=4) as ospool,
                    tc.tile_pool(name="op_ps", bufs=2, space="PSUM") as opsp,
                ):
                    wo_sb = owpool.tile([P, n_ht, dpc], MD, tag="wo")
                    nc.sync.dma_start(wo_sb[:], woT_t[:])

                    def prefetch_ct(b):
                        cts = []
                        for pp in range(2):
                            ct_t = ct_b[b][pp].ap().rearrange(
                                "(t p) s -> p t s", p=P)
                            for t in range(n_ht // 2):
                                c_t = ctpool.tile([P, SQ], MD, tag="ct")
                                nc.sync.dma_start(c_t[:], ct_t[:, t, :])
                                cts.append(c_t)
                        return cts

                    def emit_oproj(b, cts):
                        for st in range(SQ // P):
                            ps = opsp.tile([P, dpc], F32, tag="op",
                                           name="op_ps")
                            for t in range(n_ht):
                                nc.tensor.matmul(
                                    ps[:],
                                    cts[t][:, st * P:(st + 1) * P],
                                    wo_sb[:, t, :],
                                    start=(t == 0), stop=(t == n_ht - 1))
                            ob = ospool.tile([P, dpc], F32, tag="ostage",
                                             name="ostage")
                            nc.scalar.copy(ob[:], ps[:])
                            nc.sync.dma_start(
                                out_cols.ap()[b * SQ + st * P:
                                              b * SQ + (st + 1) * P, :],
                                ob[:])

                    # o_proj(b) runs two PE slots after att(b), so each
                    # block's AllGather + ct prefetch hides behind ~67us of
                    # other PE work; att(0) is cheap and runs early enough
                    # that no gather is ever the critical path. Prefetch
                    # emission order respects sync-queue head-of-line
                    # blocking (each prefetch waits its gather in-queue).
                    cts1 = prefetch_ct(1)
                    att_block(2)
                    att_block(0)
                    emit_oproj(1, cts1)
                    att_block(3)
                    cts2 = prefetch_ct(2)
                    cts0 = prefetch_ct(0)
                    emit_oproj(2, cts2)
                    cts3 = prefetch_ct(3)
                    emit_oproj(0, cts0)
                    emit_oproj(3, cts3)

    nc.compile()
    return nc


def make_in_maps(hidden_states, attention_mask, w_pack, w_o):
    B, S, H = hidden_states.shape
    hpc = NUM_HEADS // N_CORES
    dpc = hpc * HEAD_DIM
    np_md = mybir.dt.np(_mm_dtype(MM_MODE))
    xT = np.ascontiguousarray(hidden_states[0].T).astype(np_md)
    # triangular mask strip for diagonal score tiles (scoresT layout:
    # rows=s_k, cols=s_q; masked where k > q -> strictly lower triangle)
    tri = np.tril(np.full((P, P), np.finfo(np.float32).min,
                          dtype=np.float32), k=-1)
    # w_o rows permuted to match the head-pair AllGather layout:
    # [pp][rank][head-in-pair] blocks of 128
    perm = np.concatenate(
        [np.arange(128 * (4 * r + 2 * pp + hh),
                   128 * (4 * r + 2 * pp + hh) + 128)
         for pp in (0, 1) for r in range(N_CORES) for hh in (0, 1)])
    in_maps = []
    for c in range(N_CORES):
        sl = slice(c * dpc, (c + 1) * dpc)
        wqk_c = np.concatenate(
            [w_pack[0 * H:1 * H][sl], w_pack[1 * H:2 * H][sl]], axis=0)
        woT_c = np.ascontiguousarray(w_o[sl].T)[perm]
        in_maps.append({
            "xT": xT,
            "wqkT": np.ascontiguousarray(wqk_c.T).astype(np_md),
            "wvT": np.ascontiguousarray(w_pack[2 * H:3 * H][sl].T
                                        ).astype(np_md),
            "tri": tri,
            "woT": np.ascontiguousarray(woT_c).astype(np_md),
        })
    return in_maps, tri


def kernel(hidden_states, attention_mask, w_pack, w_o):
    B, S, H = hidden_states.shape
    assert B == 1 and H == NUM_HEADS * HEAD_DIM
    assert S % (2 * SQ) == 0

    # the kernel hardcodes the causal structure; verify the mask matches
    mask = np.asarray(np.broadcast_to(attention_mask, (1, 1, S, S))[0, 0],
                      dtype=np.float32)
    assert np.all(np.tril(mask) == 0.0), "mask must be causal"
    assert np.all(mask[np.triu_indices(S, 1)] <= -1e30), "mask must be causal"

    in_maps, _ = make_in_maps(hidden_states, attention_mask, w_pack, w_o)

    key = (S, H, MM_MODE)
    if key not in _CACHE:
        _CACHE[key] = build(S, H, MM_MODE)
    nc = _CACHE[key]

    res = run_bass_kernel_spmd(nc, in_maps, core_ids=list(range(N_CORES)))
    out = np.concatenate(
        [res.results[c]["out_cols"] for c in range(N_CORES)], axis=1)
    return out.reshape(1, S, H).astype(np.float32)


# revision 35
# speedup vs baseline: 1.0268x; 1.0268x over previous
"""Baichuan attention on 8 Trainium2 NeuronCores — tensor-parallel over heads.

Sharding: core c computes heads [4c, 4c+4): its slice of the fused QKV
projection, attention for those heads, then 1/8 of o_proj's output columns
after an AllGather of the per-core context slices (moves 4MB/rank instead of
a 32MB AllReduce of partial sums; mathematically identical to the module's
world_size logic).

Schedule (v2): attention blocks are interleaved into the QKV phase so the
collectives and o_proj always have PE work to hide behind:

    half0-QKV | att(1) | half1-QKV | att(2) op(1) att(3) op(2)
                                     att(0) op(3) op(0)

att(1) runs between the QKV halves (hiding the half1 x-prefetch), att(0) is
deferred to the end so the final o_proj's AllGather completes while two other
o_proj blocks stream — the serial AllGather+o_proj tail of v1 disappears.

Causal structure: diagonal score tiles are narrowed to their live columns
(moving width 512-128j) and masked with a single resident 128x128 triangular
constant instead of a 4MB mask DMA. Score tiles are computed in pairs into
2-bank PSUM tiles so one exp activation covers both (halving ACT dispatch
overhead); av/rowsum matmuls lag one slot behind scores, which hides the
scores->exp->av chain latency at a steady state of zero PE bubbles.

Matmul operands are fp16 (1 cyc/row on the PE) with fp32 PSUM accumulation.
"""

import numpy as np

import concourse.bacc as bacc
import concourse.bass_isa as bass_isa
import concourse.mybir as mybir
import concourse.tile as tile
from concourse.bass_utils import run_bass_kernel_spmd

F32 = mybir.dt.float32

N_CORES = 8
NUM_HEADS = 32
HEAD_DIM = 128
P = 128          # SBUF partitions / PE contraction tile
SQ = 512         # s_q block width (PSUM bank = 512 fp32)
MM_MODE = "f16"  # 'f16' | 'f32' (operand dtype for matmuls)

_CACHE: dict = {}


def _mm_dtype(mode):
    return {"f16": mybir.dt.float16, "f32": F32}[mode]


def build(S, H, mode=MM_MODE):
    MD = _mm_dtype(mode)
    hpc = NUM_HEADS // N_CORES          # heads per core
    dpc = hpc * HEAD_DIM                # per-core slice of the hidden dim
    n_ht = H // P                       # contraction tiles for QKV/o_proj
    n_qk = 2 * dpc // P                 # q+k output tiles
    n_sq = S // SQ                      # s_q blocks
    n_st = S // P                       # s_k tiles
    scale = 1.0 / np.sqrt(np.float32(HEAD_DIM))
    s_half = S // 2
    sb_per_half = s_half // SQ
    diag_per_b = SQ // P                # diagonal k-tiles per s_q block

    nc = bacc.Bacc("TRN2", target_bir_lowering=False, debug=False,
                   num_devices=N_CORES)

    xT = nc.dram_tensor("xT", [H, S], MD, kind="ExternalInput")
    wqkT = nc.dram_tensor("wqkT", [H, 2 * dpc], MD, kind="ExternalInput")
    wvT = nc.dram_tensor("wvT", [H, dpc], MD, kind="ExternalInput")
    tri = nc.dram_tensor("tri", [P, P], F32, kind="ExternalInput")
    woT = nc.dram_tensor("woT", [H, dpc], MD, kind="ExternalInput")
    out_cols = nc.dram_tensor("out_cols", [S, dpc], F32, kind="ExternalOutput")

    # AllGather in head-pair chunks: gat[b][pp] holds local heads
    # {2pp, 2pp+1} for s_q block b; ct[b][pp] gathers those pairs from all
    # ranks. o_proj consumes them against host-permuted w_o rows.
    gat_b = [[nc.dram_tensor(f"gat_{b}_{pp}", [dpc // 2, SQ], MD)
              for pp in range(2)] for b in range(n_sq)]
    ct_b = [[nc.dram_tensor(f"ct_{b}_{pp}", [H // 2, SQ], MD,
                            addr_space="Shared") for pp in range(2)]
            for b in range(n_sq)]

    xT_t = xT.ap().rearrange("(t p) s -> p t s", p=P)
    wqkT_t = wqkT.ap().rearrange("(t p) o -> p t o", p=P)
    wvT_t = wvT.ap().rearrange("(t p) o -> p t o", p=P)
    woT_t = woT.ap().rearrange("(t p) j -> p t j", p=P)

    with tile.TileContext(nc) as tc:
        with (
            tc.tile_pool(name="consts", bufs=1) as cpool,
            tc.tile_pool(name="span", bufs=1) as span,
            tc.tile_pool(name="qkv_wv", bufs=1) as wvpool,
            tc.tile_pool(name="at_exp", bufs=3) as epool,
            tc.tile_pool(name="at_out", bufs=2) as opool,
            tc.tile_pool(name="at_r", bufs=2) as rpool,
        ):
            ones_s = cpool.tile([P, P], F32, tag="ones_s")
            nc.gpsimd.memset(ones_s[:], 1.0)
            ones_f = cpool.tile([P, P], mybir.dt.float32r, tag="ones_f")
            nc.vector.tensor_copy(ones_f[:], ones_s[:])
            tri_t = cpool.tile([P, P], F32, tag="tri")
            nc.sync.dma_start(tri_t[:], tri.ap()[:, :])

            # v ([s_k, d] natural, all heads) and q/k (transposed, all heads)
            # live in SBUF for the whole kernel; QKV evictions write them
            # directly (no DRAM bounce)
            v_sb = span.tile([P, n_st, dpc], MD, tag="v")
            qk_all = span.tile([P, n_qk, S], MD, tag="qk")
            wv_sb = wvpool.tile([P, n_ht, dpc], MD, tag="wv")

            # =============== QKV projection (one half of S) ===============
            # q/k transposed into resident qk_all; v natural into v_sb.
            def qkv_half(half, xpool, wpool, pspool):
                w_tiles = {}
                w_tiles[0] = wpool.tile([P, n_ht, P], MD, tag="w",
                                        name="w_tile")
                nc.sync.dma_start(w_tiles[0][:], wqkT_t[:, :, 0 * P:1 * P])
                # x arrives in 8-t-tile chunks: batching descriptors keeps
                # the sync queue's ~0.7us/descriptor issue rate off the
                # critical path (64 fine-grained descriptors took ~45us to
                # issue, starving the first ot pass)
                XCH = 8
                xq = []
                for sb in range(sb_per_half):
                    x_tile = xpool.tile([P, n_ht, SQ], MD, tag="x",
                                        bufs=2, name="x_tile")
                    lo = half * s_half + sb * SQ
                    for c in range(n_ht // XCH):
                        nc.sync.dma_start(
                            x_tile[:, c * XCH:(c + 1) * XCH, :],
                            xT_t[:, c * XCH:(c + 1) * XCH, lo:lo + SQ])
                    xq.append(x_tile)
                for ot in range(n_qk):
                    if ot not in w_tiles:
                        w_tiles[ot] = wpool.tile([P, n_ht, P], MD,
                                                 tag="w", name="w_tile")
                        nc.sync.dma_start(
                            w_tiles[ot][:],
                            wqkT_t[:, :, ot * P:(ot + 1) * P])
                    w_tile = w_tiles[ot]
                    # wv is first needed ~134us into half0 (the v phase);
                    # issuing it at ot==5 keeps its 4MB out of the
                    # bandwidth-starved startup window
                    if half == 0 and ot == 5:
                        nc.sync.dma_start(wv_sb[:], wvT_t[:])
                    if half == 0:
                        # t-outer with both quarters' psums open: the first
                        # ot pass consumes x chunks in DMA-arrival order
                        # instead of draining quarter 0 first, which halves
                        # the startup underrun while x is still streaming
                        # in. Needs 4 psum bufs, so half1 (2 bufs, no
                        # startup problem) keeps the sb-inner form.
                        ps_q = [pspool.tile([P, SQ], F32, tag="qkv",
                                            name=f"ps_q{sb}")
                                for sb in range(sb_per_half)]
                        for t in range(n_ht):
                            for sb in range(sb_per_half):
                                nc.tensor.matmul(
                                    ps_q[sb][:],
                                    w_tile[:, t, :],
                                    xq[sb][:, t, :],
                                    start=(t == 0), stop=(t == n_ht - 1))
                        for sb in range(sb_per_half):
                            # fold the softmax scale into q at eviction
                            mul = scale if ot < dpc // P else 1.0
                            lo = half * s_half + sb * SQ
                            nc.scalar.mul(qk_all[:, ot, lo:lo + SQ],
                                          ps_q[sb][:], mul)
                    else:
                        for sb in range(sb_per_half):
                            ps = pspool.tile([P, SQ], F32, tag="qkv")
                            for t in range(n_ht):
                                nc.tensor.matmul(
                                    ps[:],
                                    w_tile[:, t, :],
                                    xq[sb][:, t, :],
                                    start=(t == 0), stop=(t == n_ht - 1))
                            mul = scale if ot < dpc // P else 1.0
                            lo = half * s_half + sb * SQ
                            nc.scalar.mul(qk_all[:, ot, lo:lo + SQ],
                                          ps[:], mul)
                # v: psum [s=128, dpc] accumulated over h-tiles
                for sti in range(s_half // P):
                    st_g = half * (s_half // P) + sti
                    sb, off = (sti * P) // SQ, (sti * P) % SQ
                    ps_v = pspool.tile([P, dpc], F32, tag="qkv")
                    for t in range(n_ht):
                        nc.tensor.matmul(
                            ps_v[:],
                            xq[sb][:, t, off:off + P],
                            wv_sb[:, t, :],
                            start=(t == 0), stop=(t == n_ht - 1))
                    nc.vector.tensor_copy(v_sb[:, st_g, :], ps_v[:])

            with (
                tc.tile_pool(name="qkv_x0", bufs=1) as xpool0,
                tc.tile_pool(name="qkv_w0", bufs=3) as wpool0,
                tc.tile_pool(name="qkv_ps0", bufs=4, space="PSUM") as psp0,
            ):
                qkv_half(0, xpool0, wpool0, psp0)

            # attention PSUM pools: pss 2x2 banks + out 1 + row 1 = 6 banks,
            # leaving 2 for the half1 QKV pool / later the o_proj pool
            with tc.tile_pool(name="at_ps", bufs=1, space="PSUM") as aps:

                # ======== attention for s_q block b (4 local heads) ========
                # scores are computed transposed (scoresT[k, q]) so the PE
                # contraction dim sits on partitions for every matmul.
                # Diagonal tiles are narrowed to columns [128j, 512) and get
                # the triangular mask strip added in place. Tiles are
                # processed in slots of two; exp of slot k runs while the PE
                # streams slot k+1's scores, and av/rowsum of slot k follow —
                # steady state has no PE bubble.
                def att_block(b):
                    q_lo = b * SQ
                    n_full = b * diag_per_b
                    slots = []
                    for g in range(n_full // 2):
                        slots.append([(2 * g, 0), (2 * g + 1, 0)])
                    for g in range(diag_per_b // 2):
                        slots.append([(n_full + 2 * g, 2 * g),
                                      (n_full + 2 * g + 1, 2 * g + 1)])
                    n_mm = n_full + diag_per_b
                    for h in range(hpc):
                        ps_o = aps.tile([P, SQ], F32, tag="out",
                                        name="ps_o")
                        # softmax denominator: accumulate exp tiles on the
                        # vector engine (fp16 adds into an SBUF fp32
                        # accumulator) and cross-partition-sum on the idle
                        # gpsimd — frees ~36us of PE streaming vs the
                        # all-ones rowsum matmul, and frees a PSUM bank
                        acc = rpool.tile([P, SQ], mybir.dt.float32r,
                                         tag="acc", name="acc")
                        mi = 0

                        def emit_av(pend):
                            nonlocal mi
                            slot, ex2 = pend
                            for ii, (t, j) in enumerate(slot):
                                lo = P * j
                                first, last = mi == 0, mi == n_mm - 1
                                nc.tensor.matmul(
                                    ps_o[:, lo:SQ],
                                    v_sb[:, t, h * P:(h + 1) * P],
                                    ex2[:, ii, lo:SQ],
                                    start=first, stop=last,
                                    skip_group_check=True)
                                if first:
                                    # first tile is always full-width
                                    nc.vector.tensor_copy(
                                        acc[:], ex2[:, ii, :])
                                else:
                                    nc.vector.tensor_add(
                                        acc[:, lo:SQ], acc[:, lo:SQ],
                                        ex2[:, ii, lo:SQ])
                                mi += 1

                        # av/rowsum lag scores by TWO slots: ps2 is freed by
                        # the exp read (not by av), so depth-2 costs no extra
                        # PSUM and the scores->exp->av chain latency is fully
                        # hidden even at block starts — the PE stream has no
                        # bubble for the scheduler to (mis)fill with o_proj
                        # work whose ct tiles aren't on-chip yet.
                        pend = []
                        for slot in slots:
                            ps2 = aps.tile([P, 2, SQ], F32, tag="pss",
                                           bufs=2, name="ps2")
                            ex2 = epool.tile([P, 2, SQ], MD, tag="exp",
                                             name="ex2")
                            is_diag = slot[0][0] >= n_full
                            for ii, (t, j) in enumerate(slot):
                                lo = P * j
                                nc.tensor.matmul(
                                    ps2[:, ii, lo:SQ],
                                    qk_all[:, hpc + h, t * P:(t + 1) * P],
                                    qk_all[:, h, q_lo + lo:q_lo + SQ],
                                    start=True, stop=True)
                                if is_diag:
                                    nc.vector.tensor_add(
                                        ps2[:, ii, lo:lo + P],
                                        ps2[:, ii, lo:lo + P], tri_t[:])
                            if is_diag:
                                for ii, (t, j) in enumerate(slot):
                                    lo = P * j
                                    nc.scalar.activation(
                                        ex2[:, ii, lo:SQ], ps2[:, ii, lo:SQ],
                                        mybir.ActivationFunctionType.Exp)
                            else:
                                nc.scalar.activation(
                                    ex2[:, :, :], ps2[:, :, :],
                                    mybir.ActivationFunctionType.Exp)
                            pend.append((slot, ex2))
                            if len(pend) > 2:
                                emit_av(pend.pop(0))
                        for p in pend:
                            emit_av(p)

                        # cross-partition sum of the exp accumulator in ONE
                        # 512-wide f32r ones-matmul (0.26us) — vs per-tile
                        # rowsum matmuls this cuts the PE's denominator cost
                        # ~8x; f32r truncation (FP22) adds only ~6e-5 rel
                        # error to the denominator
                        ps_row = aps.tile([P, SQ], F32, tag="row",
                                          name="ps_row")
                        nc.tensor.matmul(
                            ps_row[:], ones_f[:], acc[:],
                            start=True, stop=True)
                        recip = rpool.tile([P, SQ], F32, tag="recip",
                                           name="recip")
                        nc.vector.reciprocal_approx_fast(recip[:], ps_row[:])
                        ob = opool.tile([P, SQ], MD, tag="ob", name="ob")
                        nc.vector.tensor_mul(ob[:], ps_o[:], recip[:])
                        nc.sync.dma_start(
                            gat_b[b][h // 2].ap()[(h % 2) * P:
                                                  (h % 2 + 1) * P, :], ob[:])
                        if h % 2 == 1:
                            nc.gpsimd.collective_compute(
                                "AllGather", mybir.AluOpType.bypass,
                                replica_groups=[list(range(N_CORES))],
                                ins=[gat_b[b][h // 2].ap().opt()],
                                outs=[ct_b[b][h // 2].ap().opt()])

                # att(1) fits between the QKV halves: it needs only the
                # first-half k/v/q, and its PE work hides the half1 x DMA
                with (
                    tc.tile_pool(name="qkv_x1", bufs=1) as xpool1,
                    tc.tile_pool(name="qkv_w1", bufs=3) as wpool1,
                    tc.tile_pool(name="qkv_ps1", bufs=2,
                                 space="PSUM") as psp1,
                ):
                    att_block(1)
                    qkv_half(1, xpool1, wpool1, psp1)

                with (
                    tc.tile_pool(name="op_w", bufs=1) as owpool,
                    tc.tile_pool(name="op_ct", bufs=34) as ctpool,
                    tc.tile_pool(name="op_stage", bufs=4) as ospool,
                    tc.tile_pool(name="op_ps", bufs=2, space="PSUM") as opsp,
                ):
                    wo_sb = owpool.tile([P, n_ht, dpc], MD, tag="wo")
                    nc.sync.dma_start(wo_sb[:], woT_t[:])

                    def prefetch_ct(b):
                        cts = []
                        for pp in range(2):
                            ct_t = ct_b[b][pp].ap().rearrange(
                                "(t p) s -> p t s", p=P)
                            for t in range(n_ht // 2):
                                c_t = ctpool.tile([P, SQ], MD, tag="ct")
                                nc.sync.dma_start(c_t[:], ct_t[:, t, :])
                                cts.append(c_t)
                        return cts

                    def emit_oproj(b, cts):
                        for st in range(SQ // P):
                            ps = opsp.tile([P, dpc], F32, tag="op",
                                           name="op_ps")
                            for t in range(n_ht):
                                nc.tensor.matmul(
                                    ps[:],
                                    cts[t][:, st * P:(st + 1) * P],
                                    wo_sb[:, t, :],
                                    start=(t == 0), stop=(t == n_ht - 1))
                            ob = ospool.tile([P, dpc], F32, tag="ostage",
                                             name="ostage")
                            nc.scalar.copy(ob[:], ps[:])
                            nc.sync.dma_start(
                                out_cols.ap()[b * SQ + st * P:
                                              b * SQ + (st + 1) * P, :],
                                ob[:])

                    # o_proj(b) runs two PE slots after att(b), so each
                    # block's AllGather + ct prefetch hides behind ~67us of
                    # other PE work; att(0) is cheap and runs early enough
                    # that no gather is ever the critical path. Prefetch
                    # emission order respects sync-queue head-of-line
                    # blocking (each prefetch waits its gather in-queue).
                    cts1 = prefetch_ct(1)
                    att_block(2)
                    att_block(0)
                    emit_oproj(1, cts1)
                    att_block(3)
                    cts2 = prefetch_ct(2)
                    cts0 = prefetch_ct(0)
                    emit_oproj(2, cts2)
                    cts3 = prefetch_ct(3)
                    emit_oproj(0, cts0)
                    emit_oproj(3, cts3)

    nc.compile()
    return nc


def make_in_maps(hidden_states, attention_mask, w_pack, w_o):
    B, S, H = hidden_states.shape
    hpc = NUM_HEADS // N_CORES
    dpc = hpc * HEAD_DIM
    np_md = mybir.dt.np(_mm_dtype(MM_MODE))
    xT = np.ascontiguousarray(hidden_states[0].T).astype(np_md)
    # triangular mask strip for diagonal score tiles (scoresT layout:
    # rows=s_k, cols=s_q; masked where k > q -> strictly lower triangle)
    tri = np.tril(np.full((P, P), np.finfo(np.float32).min,
                          dtype=np.float32), k=-1)
    # w_o rows permuted to match the head-pair AllGather layout:
    # [pp][rank][head-in-pair] blocks of 128
    perm = np.concatenate(
        [np.arange(128 * (4 * r + 2 * pp + hh),
                   128 * (4 * r + 2 * pp + hh) + 128)
         for pp in (0, 1) for r in range(N_CORES) for hh in (0, 1)])
    in_maps = []
    for c in range(N_CORES):
        sl = slice(c * dpc, (c + 1) * dpc)
        wqk_c = np.concatenate(
            [w_pack[0 * H:1 * H][sl], w_pack[1 * H:2 * H][sl]], axis=0)
        woT_c = np.ascontiguousarray(w_o[sl].T)[perm]
        in_maps.append({
            "xT": xT,
            "wqkT": np.ascontiguousarray(wqk_c.T).astype(np_md),
            "wvT": np.ascontiguousarray(w_pack[2 * H:3 * H][sl].T
                                        ).astype(np_md),
            "tri": tri,
            "woT": np.ascontiguousarray(woT_c).astype(np_md),
        })
    return in_maps, tri


def kernel(hidden_states, attention_mask, w_pack, w_o):
    B, S, H = hidden_states.shape
    assert B == 1 and H == NUM_HEADS * HEAD_DIM
    assert S % (2 * SQ) == 0

    # the kernel hardcodes the causal structure; verify the mask matches
    mask = np.asarray(np.broadcast_to(attention_mask, (1, 1, S, S))[0, 0],
                      dtype=np.float32)
    assert np.all(np.tril(mask) == 0.0), "mask must be causal"
    assert np.all(mask[np.triu_indices(S, 1)] <= -1e30), "mask must be causal"

    in_maps, _ = make_in_maps(hidden_states, attention_mask, w_pack, w_o)

    key = (S, H, MM_MODE)
    if key not in _CACHE:
        _CACHE[key] = build(S, H, MM_MODE)
    nc = _CACHE[key]

    res = run_bass_kernel_spmd(nc, in_maps, core_ids=list(range(N_CORES)))
    out = np.concatenate(
        [res.results[c]["out_cols"] for c in range(N_CORES)], axis=1)
    return out.reshape(1, S, H).astype(np.float32)
